# revision 29
# baseline (speedup 1.0000x reference)
"""Linear attention (elu(x)+1 feature map) Bass/Tile kernel for Trainium2.

Problem: B=4, H=16, S=4096, D=64, fp32.
  Qf = elu(Q)+1; Kf = elu(K)+1
  KV = Kf^T (V*mask);  Ksum = Kf^T mask
  out = (Qf @ KV) / (Qf . Ksum)

Sharding: 64 (b,h) pairs data-parallel over 8 cores, 8 pairs/core, no
collectives. Pairs processed in 4 groups of 2 so matmuls use full 128
partitions.

v14 design (timeline: v1 384 -> v5 86 -> v6 109 -> v7 112 -> v8 101 ->
v9 91 -> v10 84 -> v13 83 -> v14 80):
Measured HW rates that drove the design: DVE tensor_scalar gets the 4x
mode (~0.33ns/el), tensor_tensor only 2x (~0.59), PSUM-fp32 reads 1x;
ACT ~0.9ns/el (so ACT affords only exp/relu table ops); the XBAR
DMA-transpose occupies its HWDGE queue 4-8us/MiB (unusable). Steady
state is DVE+ACT dual-bound at ~12-13us/group vs the ~11.9us/group HBM
wall; the rest of the span is ramp + drain, attacked separately.
- K path: exp on ACT, relu/min as 4x DVE tensor_scalars; the final
  kf = mn+rl ADD runs on the PE by double-pumping the KV accumulation
  (KV = sum mnk x v + sum rlk x v, fp32 accumulate).
- KV matmuls are 130-col pair-merged over [K0|K1] x [V0|m0|V1|m1]
  (cross-pair blocks land in unused PSUM columns).
- Q path: PE-transpose raw Q (bf16 PSUM), then exp/relu as one 1024-el
  ACT op each per block (they double as the PSUM->SBUF evacuation),
  min on DVE TS (4x), add on DVE TT (2x). Group 0's relu runs on DVE
  instead (lead-in is ACT-bound while DVE idles).
- Phase B: den matmuls separate ([128,2] ks2 rhs) so the out PSUM tile
  is [128, NJ, 128] fp32 = 2 banks and DOUBLE-buffers: the next
  block's matmuls overlap this block's normalize, which collapses the
  end-of-kernel drain from ~13us to ~7us. One reciprocal + one
  broadcast-TT multiply per block evacuate PSUM; blocks 2/3 store
  per-block to shorten the tail.
- bd/ks2 extraction runs on ACT (Copy) in its group-boundary idle gaps.
- All six dmas of a group issue together, two groups ahead (load_a);
  group 0's first half is split per-block so compute starts ~2us in.
- PSUM: tp 2x1 + kv 1 + ob 2x2 + dn 1 = 8 banks exactly.
- s-interleave s = blk*1024 + p*8 + j everywhere (2KiB DMA runs);
  output stored pair-interleaved [s, u, d], deinterleaved on host.
- Queues: sync HWDGE = K loads + O stores; gpsimd SWDGE = Q + V loads;
  scalar queue = ACT compute only.
"""

import numpy as np

import concourse.bass as bass
import concourse.mybir as mybir
import concourse.tile as tile
from concourse.bass_utils import run_bass_kernel_spmd
from concourse.masks import make_identity

F32 = mybir.dt.float32
BF16 = mybir.dt.bfloat16
AF = mybir.ActivationFunctionType
ALU = mybir.AluOpType

N_CORES = 8
PAIRS = 8          # (b,h) pairs per core
S = 4096
D = 64
E = D + 1          # V is host-padded with the mask column
E2 = 2 * E         # both pairs' V columns in one rhs
NB = 4             # blocks (of 1024 rows) per pair
NJ = 8             # s = blk*1024 + p*8 + j
NG = PAIRS // 2    # pair-groups
NH = 2             # half-groups (2 blocks each) per group
OBW = 256          # fp32 cols per j-slot in the out PSUM tile (bank padding)


def build_bass() -> bass.Bass:
    from contextlib import ExitStack
    from concourse.bacc import Bacc
    nc = Bacc()
    Qh = nc.dram_tensor("Q", [NG, S, 2, D], BF16, kind="ExternalInput")
    Kh = nc.dram_tensor("K", [NG, S, 2, D], BF16, kind="ExternalInput")
    Vh = nc.dram_tensor("V", [NG, S, 2, E], BF16, kind="ExternalInput")
    Oh = nc.dram_tensor("O", [NG, S, 2, D], BF16, kind="ExternalOutput")

    # s = h*2048 + c*1024 + p*8 + j
    Qv = [Qh[g].rearrange("(h c p j) u d -> h p c j u d",
                          h=NH, c=2, p=128, j=NJ) for g in range(NG)]
    Kv = [Kh[g].rearrange("(h c p j) u d -> h p c j u d",
                          h=NH, c=2, p=128, j=NJ) for g in range(NG)]
    Vv = [Vh[g].rearrange("(b p j) u e -> p b j u e", b=NB, p=128, j=NJ)
          for g in range(NG)]
    Ov = [Oh[g].rearrange("(b p j) u d -> p b j u d", b=NB, p=128, j=NJ)
          for g in range(NG)]

    with tile.TileContext(nc) as tc, ExitStack() as ctx, \
            nc.allow_low_precision("bf16 pipeline; fro gate is 2e-2"):
        consts = ctx.enter_context(tc.tile_pool(name="consts", bufs=1))
        kr_pool = ctx.enter_context(tc.tile_pool(name="kr", bufs=4))
        exk_pool = ctx.enter_context(tc.tile_pool(name="exk", bufs=2))
        rlk_pool = ctx.enter_context(tc.tile_pool(name="rlk", bufs=2))
        mnk_pool = ctx.enter_context(tc.tile_pool(name="mnk", bufs=2))
        vm_pool = ctx.enter_context(tc.tile_pool(name="vm", bufs=2))
        qr_pool = ctx.enter_context(tc.tile_pool(name="qr", bufs=4))
        exq_pool = ctx.enter_context(tc.tile_pool(name="exq", bufs=2))
        rlq_pool = ctx.enter_context(tc.tile_pool(name="rlq", bufs=2))
        mnq_pool = ctx.enter_context(tc.tile_pool(name="mnq", bufs=2))
        qtf_pool = ctx.enter_context(tc.tile_pool(name="qtf", bufs=2))
        bd_pool = ctx.enter_context(tc.tile_pool(name="bd", bufs=2))
        rec_pool = ctx.enter_context(tc.tile_pool(name="rec", bufs=2))
        ks_pool = ctx.enter_context(tc.tile_pool(name="ks", bufs=2))
        osb_pool = ctx.enter_context(tc.tile_pool(name="osb", bufs=2))
        tp_psum = ctx.enter_context(tc.tile_pool(name="tpps", bufs=2, space="PSUM"))
        kv_psum = ctx.enter_context(tc.tile_pool(name="kvps", bufs=1, space="PSUM"))
        ob_psum = ctx.enter_context(tc.tile_pool(name="obps", bufs=2, space="PSUM"))
        dn_psum = ctx.enter_context(tc.tile_pool(name="dnps", bufs=1, space="PSUM"))

        identity = consts.tile([128, 128], BF16)
        make_identity(nc, identity)

        kv_ps_g = [None] * NG
        qtf_g = [None] * NG
        bd_g = [None] * NG
        ks2_g = [None] * NG

        def load_a(g):
            # allocate group tiles and issue all 6 dmas; group 0's first
            # half is split per-block so compute can start ~2us earlier
            kv_ps = kv_psum.tile([128, E2], F32, tag="kv", name=f"kv_{g}")
            kv_ps_g[g] = kv_ps
            vm = vm_pool.tile([128, NB, NJ, 2, E], BF16, tag="vm",
                              name=f"vm_{g}")
            qtf = qtf_pool.tile([128, NB, NJ, 128], BF16, tag="qtf",
                                name=f"qtf_{g}")
            qtf_g[g] = qtf
            kraw, qraw = [], []
            for h in range(NH):
                b0 = 2 * h
                kraw.append(kr_pool.tile([128, 2, NJ, 2, D], BF16, tag="kr",
                                         name=f"kr_{g}_{h}"))
                qraw.append(qr_pool.tile([128, 2, NJ, 2, D], BF16, tag="qr",
                                         name=f"qr_{g}_{h}"))
                if g == 0 and h == 0:
                    for c in range(2):
                        nc.sync.dma_start(out=kraw[h][:, c],
                                          in_=Kv[g][h][:, c])
                        nc.gpsimd.dma_start(out=qraw[h][:, c],
                                            in_=Qv[g][h][:, c])
                        nc.gpsimd.dma_start(out=vm[:, b0 + c],
                                            in_=Vv[g][:, b0 + c])
                else:
                    nc.sync.dma_start(out=kraw[h], in_=Kv[g][h])
                    nc.gpsimd.dma_start(out=qraw[h], in_=Qv[g][h])
                    nc.gpsimd.dma_start(out=vm[:, b0:b0 + 2],
                                        in_=Vv[g][:, b0:b0 + 2])
            return kv_ps, vm, qtf, kraw, qraw

        def phase_a(g, tiles):
            # generator: 4 yields (one per block)
            kv_ps, vm, qtf, kraws, qraws = tiles

            for h in range(NH):
                b0 = 2 * h
                kraw, qraw = kraws[h], qraws[h]

                # Kf = elu(K)+1 = min(exp(K),1) + relu(K); the add happens
                # on the PE via double-pumped KV accumulation below.
                exk = exk_pool.tile([128, 2, NJ, 2, D], BF16, tag="exk",
                                    name=f"exk_{g}_{h}")
                rlk = rlk_pool.tile([128, 2, NJ, 2, D], BF16, tag="rlk",
                                    name=f"rlk_{g}_{h}")
                mnk = mnk_pool.tile([128, 2, NJ, 2, D], BF16, tag="mnk",
                                    name=f"mnk_{g}_{h}")
                if g == 0 and h == 0:
                    # per-block ops so the quarter-split dmas unblock early
                    for c in range(2):
                        nc.scalar.activation(exk[:, c], kraw[:, c], AF.Exp)
                        nc.vector.tensor_scalar_max(rlk[:, c], kraw[:, c],
                                                    0.0)
                        nc.vector.tensor_scalar_min(mnk[:, c], exk[:, c],
                                                    1.0)
                else:
                    nc.scalar.activation(exk, kraw, AF.Exp)
                    nc.vector.tensor_scalar_max(rlk, kraw, 0.0)
                    nc.vector.tensor_scalar_min(mnk, exk, 1.0)

                for c in range(2):
                    blk = b0 + c
                    # transpose raw Q -> [(u,d), s] (bf16 PSUM)
                    tp = tp_psum.tile([128, NJ, 128], BF16, tag="tp",
                                      name=f"tp_{g}_{blk}")
                    for j in range(NJ):
                        nc.tensor.transpose(tp[:, j], qraw[:, c, j],
                                            identity)
                    # Qf = min(exp,1)+relu; ACT ops evacuate tp to SBUF
                    exq = exq_pool.tile([128, NJ, 128], BF16, tag="exq",
                                        name=f"exq_{g}_{blk}")
                    rlq = rlq_pool.tile([128, NJ, 128], BF16, tag="rlq",
                                        name=f"rlq_{g}_{blk}")
                    mnq = mnq_pool.tile([128, NJ, 128], BF16, tag="mnq",
                                        name=f"mnq_{g}_{blk}")
                    nc.scalar.activation(exq, tp, AF.Exp)
                    if g == 0:
                        # lead-in is ACT-bound while DVE idles
                        nc.vector.tensor_scalar_max(rlq, tp, 0.0)
                    else:
                        nc.scalar.activation(rlq, tp, AF.Relu)
                    nc.vector.tensor_scalar_min(mnq, exq, 1.0)
                    nc.vector.tensor_tensor(out=qtf[:, blk], in0=mnq,
                                            in1=rlq, op=ALU.add)

                    # KV accumulation, double-pumped: mnk-pass + rlk-pass
                    for j in range(NJ):
                        cc = blk * NJ + j
                        nc.tensor.matmul(
                            kv_ps, lhsT=mnk[:, c, j], rhs=vm[:, blk, j],
                            start=(cc == 0), stop=False,
                            skip_group_check=True)
                        nc.tensor.matmul(
                            kv_ps, lhsT=rlk[:, c, j], rhs=vm[:, blk, j],
                            start=False, stop=(cc == NB * NJ - 1),
                            skip_group_check=True)
                    yield

        def extract_bd(g):
            kv_ps = kv_ps_g[g]
            # bd = block-diag [KV0|KV1]; ks2 = the two Ksum columns
            bd = bd_pool.tile([128, 128], BF16, tag="bd", name=f"bd_{g}")
            ks2 = ks_pool.tile([128, 2], BF16, tag="ks2", name=f"ks2_{g}")
            nc.gpsimd.memset(bd, 0.0)
            nc.gpsimd.memset(ks2, 0.0)
            # ACT idles at group boundaries: do the extraction there
            nc.scalar.activation(bd[0:64, 0:D], kv_ps[0:64, 0:D], AF.Copy)
            nc.scalar.activation(bd[64:128, D:128], kv_ps[64:128, E:E + D],
                                 AF.Copy)
            nc.scalar.activation(ks2[0:64, 0:1], kv_ps[0:64, D:E], AF.Copy)
            nc.scalar.activation(ks2[64:128, 1:2], kv_ps[64:128, E + D:E2],
                                 AF.Copy)
            bd_g[g], ks2_g[g] = bd, ks2

        def phase_b(g):
            bd, ks2 = bd_g[g], ks2_g[g]
            qtf = qtf_g[g]
            osb = osb_pool.tile([128, NB, NJ, 2, D], BF16, tag="osb",
                                name=f"osb_{g}")
            for blk in range(NB):
                if blk == 2:
                    nc.sync.dma_start(out=Ov[g][:, 0:2], in_=osb[:, 0:2])
                if blk > 0:
                    yield
                # ob double-buffers (2 banks each): next block's matmuls
                # overlap this block's normalize
                ob = ob_psum.tile([128, NJ, 128], F32, tag="ob",
                                  name=f"ob_{g}_{blk}")
                dn = dn_psum.tile([128, NJ, 2], F32, tag="dn",
                                  name=f"dn_{g}_{blk}")
                for j in range(NJ):
                    lhsT = qtf[:, blk, j]
                    nc.tensor.matmul(ob[:, j], lhsT=lhsT, rhs=bd,
                                     start=True, stop=True,
                                     skip_group_check=True)
                    nc.tensor.matmul(dn[:, j], lhsT=lhsT, rhs=ks2,
                                     start=True, stop=True,
                                     skip_group_check=True)
                rec = rec_pool.tile([128, 2, NJ], BF16, tag="rec",
                                    name=f"rec_{g}_{blk}")
                nc.vector.reciprocal(rec.rearrange("p u j -> p j u"), dn)
                nc.vector.tensor_tensor(
                    out=osb[:, blk],
                    in0=ob.rearrange("p j (u d) -> p j u d", u=2),
                    in1=rec.rearrange("p u j -> p j u")
                        .to_broadcast([128, NJ, 2, D]),
                    op=ALU.mult)
                if blk >= 2:
                    nc.sync.dma_start(out=Ov[g][:, blk:blk + 1],
                                      in_=osb[:, blk:blk + 1])
            yield

        # emission: weave B(g-1) block-chunks 1:1 between A(g) blocks;
        # loads run two groups ahead of compute (pool bufs allow it)
        tiles = [load_a(0), load_a(1), None, None]
        a_gens = [None] * NG
        b_gens = [None] * NG

        def run(gen):
            if gen is not None:
                next(gen, None)

        a_gens[0] = phase_a(0, tiles[0])
        for _ in range(NB):
            run(a_gens[0])
        extract_bd(0)
        tiles[2] = load_a(2)
        b_gens[0] = phase_b(0)
        for g in range(1, NG):
            a_gens[g] = phase_a(g, tiles[g])
            for blk in range(NB):
                run(a_gens[g])
                run(b_gens[g - 1])
            extract_bd(g)
            if g + 2 < NG:
                tiles[g + 2] = load_a(g + 2)
            b_gens[g] = phase_b(g)
        for _ in range(NB):
            run(b_gens[NG - 1])

    nc.finalize()
    return nc


_NC_CACHE = None


def _get_nc():
    global _NC_CACHE
    if _NC_CACHE is None:
        _NC_CACHE = build_bass()
    return _NC_CACHE


def kernel(Q: np.ndarray, K: np.ndarray, V: np.ndarray, mask: np.ndarray,
           _trace: bool = False):
    import ml_dtypes
    BF = ml_dtypes.bfloat16
    B, H = 4, 16
    NP = B * H
    per = NP // N_CORES
    ng_total = NP // 2
    # pair-interleaved bf16 host layouts: [group, s, pair, d]
    Qi = np.ascontiguousarray(
        np.asarray(Q, dtype=np.float32).reshape(ng_total, 2, S, D)
        .transpose(0, 2, 1, 3).astype(BF))
    Ki = np.ascontiguousarray(
        np.asarray(K, dtype=np.float32).reshape(ng_total, 2, S, D)
        .transpose(0, 2, 1, 3).astype(BF))
    Vr = np.asarray(V, dtype=np.float32).reshape(NP, S, D)
    Mr = np.asarray(mask, dtype=np.float32).reshape(NP, S)
    # V packed with the mask column: exact for any mask, free when ones
    Vpk = np.empty((NP, S, E), dtype=BF)
    if np.all(Mr == 1.0):
        Vpk[:, :, 0:D] = Vr
    else:
        Vpk[:, :, 0:D] = Vr * Mr[:, :, None]
    Vpk[:, :, D] = Mr
    Vi = np.ascontiguousarray(
        Vpk.reshape(ng_total, 2, S, E).transpose(0, 2, 1, 3))

    in_maps = []
    gper = per // 2
    for i in range(N_CORES):
        sl = slice(i * gper, (i + 1) * gper)
        in_maps.append({
            "Q": np.ascontiguousarray(Qi[sl]),
            "K": np.ascontiguousarray(Ki[sl]),
            "V": np.ascontiguousarray(Vi[sl]),
        })

    nc = _get_nc()
    res = run_bass_kernel_spmd(nc, in_maps, core_ids=list(range(N_CORES)),
                               trace=_trace)
    # O per core: [NGc, S, 2, D] pair-interleaved
    out = np.concatenate(
        [np.asarray(r["O"]).astype(np.float32).transpose(0, 2, 1, 3)
         .reshape(per, S, D) for r in res.results], axis=0)
    if _trace:
        kernel._last_results = res
    return out.reshape(B, H, S, D)
